# revision 38
# baseline (speedup 1.0000x reference)
"""Trainium2 Bass kernel: linear attention (softmax over feature dim) MHA.

Math (per batch m, head h):
    q = softmax_d(xq @ Wq) * D**-0.5 ; k = softmax_d(xk @ Wk) ; v = xv @ Wv
    kv_h = k_h^T @ v_h            [d, e]
    out_h = q_h @ kv_h            [n, e]
    out = concat_h(out_h) @ Wo + bo

Sharding: data-parallel over batch m (16 batches -> 2 per core, 8 cores).
No collectives. Host-side marshalling: per-core shards are uploaded as
bf16 with x tensors pre-transposed to [batch, d_model, n] so every
matmul contraction sits on the SBUF partition axis.

Device schedule (v2): a rotated software pipeline. The per-batch work is
two phases:
  pass1(b): per 128-token chunk tk: psum_k = xkT^T@Wk (tokens stationary),
    ke = exp(psum_k) [ACT], sk = per-head rowsum [DVE], rk = 1/sk [DVE],
    psum_v = xvT^T@Wv, vs = psum_v*rk [DVE], kv += vs_pair^T @ ke_pair
    (PE, 2 heads packed per 128x128 matmul, one PSUM bank for all 4 packs).
  pass2(b): W2 = blockdiag(kv)^T @ Wo (merged Wo), then per 512-token
    chunk t: psum_q = Wq^T @ xqT (features stationary), qe = exp [ACT],
    s = pool8^T@qe [PE pooling], rq = 1/s [DVE approx], bc = exp8^T@rq
    [PE broadcast], qen = qe*bc [DVE], psum_o = qen^T @ W2, DMA psum_o
    straight to DRAM (f32), bo added on host.
The For_i body runs inter(pass2(b0) braided with pass1(b1)) then
inter(pass2(b1), pass1(b0)): every q-side dependency stall (exp ->
pool -> 1/s -> bc -> qen) is bridged with k/v projection matmuls of the
other batch, keeping the PE queue dense. PSUM plan (8 banks):
mm(k/v/q):2  pso:2  kv:1  sps:1  aux(bc+w2):2.
"""

import os
import sys

for _p in ("/opt/trn_rl_repo", "/root/.axon_site/_ro/trn_rl_repo"):
    if os.path.isdir(_p) and _p not in sys.path:
        sys.path.insert(0, _p)

from contextlib import ExitStack

import ml_dtypes
import numpy as np

import concourse.mybir as mybir
import concourse.tile as tile
from concourse import bacc
from concourse.bass import ds, ts
from concourse.bass_utils import run_bass_kernel_spmd

BF16 = mybir.dt.bfloat16
F32 = mybir.dt.float32
F32R = mybir.dt.float32r
NPBF16 = ml_dtypes.bfloat16

M, N, DM = 16, 2048, 512
H, D = 8, 64
NCORES = 8
MB = M // NCORES          # batches per core
NC_DM = DM // 128         # 4 contraction chunks of 128
NT128 = N // 128          # 16 token chunks (pass 1)
NT512 = N // 512          # 4 token chunks (pass 2)
NPAIR = H // 2            # 4 head pairs

EXP = mybir.ActivationFunctionType.Exp
COPY = mybir.ActivationFunctionType.Copy

# broadcast the q softmax reciprocals with gpsimd DMAs instead of PE
# matmuls (frees 32 matmuls + 2 PSUM banks per body). Default off: the
# software-DGE path measured ~45us/body SLOWER on hardware than the PE
# broadcast matmuls despite simulating faster.
BC_VIA_DMA = os.environ.get("BC_VIA_DMA", "0") == "1"
# braid slots a chunk's kv matmuls trail behind its K/V projections
# (HW ACT/DVE latencies run longer than the cost model; 3 gives slack)
KV_DEPTH = int(os.environ.get("KV_DEPTH", "3"))
# run the per-head rowsum of ke on the (otherwise idle) gpsimd engine,
# shortening the DVE queue that feeds vs and qen
SK_GPSIMD = os.environ.get("SK_GPSIMD", "0") == "1"  # gpsimd lacks free-axis reduce
# PSUM banks: mm + pso:2 + kv:1 + sps:1 + aux = 8
MM_BUFS = 3 if BC_VIA_DMA else 2
AUX_BUFS = 1 if BC_VIA_DMA else 2


def build_program(reps: int = 1, loop_n: int = 1):
    nc = bacc.Bacc(
        "TRN2", target_bir_lowering=False, debug=False, num_devices=NCORES
    )
    xT_d = {
        n_: nc.dram_tensor(n_, [MB, DM, N], BF16, kind="ExternalInput").ap()
        for n_ in ("xqT", "xkT", "xvT")
    }
    w_dram = {
        name: nc.dram_tensor(name, [DM, DM], BF16, kind="ExternalInput").ap()
        for name in ("wq", "wk", "wv", "wo")
    }
    # pool8[p, c, h] = 1 iff h == 2c + p//64 : per-head partition pooling
    pool8_d = nc.dram_tensor("pool8", [128, NC_DM, H], BF16, kind="ExternalInput").ap()
    # exp8[h, 128c + j] = 1 iff h == 2c + j//64 : partition broadcast
    exp8_d = nc.dram_tensor("exp8", [H, DM], F32R, kind="ExternalInput").ap()
    out_d = nc.dram_tensor("out", [MB, N, DM], F32, kind="ExternalOutput").ap()

    with tile.TileContext(nc) as tc, ExitStack() as ctx:
        wpool = ctx.enter_context(tc.tile_pool(name="w", bufs=1))
        xpool = ctx.enter_context(tc.tile_pool(name="x", bufs=2))
        kepool = ctx.enter_context(tc.tile_pool(name="ke", bufs=6))
        vspool = ctx.enter_context(tc.tile_pool(name="vs", bufs=6))
        skpool = ctx.enter_context(tc.tile_pool(name="sk", bufs=6))
        kbpool = ctx.enter_context(tc.tile_pool(name="kvblk", bufs=8))
        qepool = ctx.enter_context(tc.tile_pool(name="qe", bufs=6))
        rqpool = ctx.enter_context(tc.tile_pool(name="rq", bufs=2))
        qnpool = ctx.enter_context(tc.tile_pool(name="qen", bufs=6))
        w2pool = ctx.enter_context(tc.tile_pool(name="w2", bufs=8))
        fpool = ctx.enter_context(tc.tile_pool(name="fin", bufs=8))
        bcpool = ctx.enter_context(tc.tile_pool(name="bc", bufs=4))
        pspool = ctx.enter_context(tc.tile_pool(name="ps", bufs=2, space="PSUM"))

        w_sb = {}
        for name in ("wq", "wk", "wv", "wo"):
            t = wpool.tile([128, NC_DM, DM], BF16, tag=name, name=name)
            nc.sync.dma_start(
                out=t[:, :, :],
                in_=w_dram[name].rearrange("(c p) f -> p c f", p=128),
            )
            w_sb[name] = t
        pool8_sb = wpool.tile([128, NC_DM, H], BF16, tag="pool8")
        nc.sync.dma_start(out=pool8_sb[:, :, :], in_=pool8_d)
        exp8_sb = wpool.tile([H, DM], F32R, tag="exp8")
        nc.sync.dma_start(out=exp8_sb[:, :], in_=exp8_d)

        # ---- persistent per-batch state (written every loop iteration;
        # persistent static tiles keep loop-carried deps expressible) ----
        x_sb = {}       # (name, b) -> persistent sbuf tile
        for b in range(MB):
            for name in ("xqT", "xkT", "xvT"):
                x_sb[(name, b)] = xpool.tile(
                    [128, NC_DM, N], BF16, tag=f"{name}{b}", bufs=1,
                    name=f"{name}{b}",
                )
        # single kv PSUM bank, alternating batches (read phase of batch b
        # strictly precedes the write phase of the next batch)
        kv_ps = pspool.tile([128, DM], F32, tag="kv", bufs=1, name="kv_ps")
        w2_sb = {}      # b -> list of 4 w2 sbuf tiles

        def dma_x(name, b):
            nc.sync.dma_start(
                out=x_sb[(name, b)][:, :, :],
                in_=xT_d[name][b].rearrange("(c p) n -> p c n", p=128),
            )

        def chunk_mm(b, tk):
            """K/V projection matmuls + exp/rowsum/recip/vs chain for one
            128-token chunk. kv matmuls are NOT emitted here (deferred two
            braid slots so the exp->rowsum->recip->vs chain finishes)."""
            ps_k = pspool.tile([128, DM], F32, tag="mm", bufs=MM_BUFS, name="ps_k")
            for c in range(NC_DM):
                nc.tensor.matmul(
                    ps_k[:, :],
                    x_sb[("xkT", b)][:, c, ts(tk, 128)],
                    w_sb["wk"][:, c, :],
                    start=(c == 0),
                    stop=(c == NC_DM - 1),
                )
            ke = kepool.tile([128, H, D], BF16, tag="ke")
            nc.scalar.activation(
                ke[:, :, :],
                ps_k[:, :].rearrange("p (h e) -> p h e", h=H),
                EXP,
            )
            sk = skpool.tile([128, H], F32, tag="sk")
            eng = nc.gpsimd if SK_GPSIMD else nc.vector
            eng.tensor_reduce(
                sk[:, :], ke[:, :, :],
                axis=mybir.AxisListType.X, op=mybir.AluOpType.add,
            )
            rk = skpool.tile([128, H], F32, tag="rk")
            nc.vector.reciprocal(rk[:, :], sk[:, :])

            ps_v = pspool.tile([128, DM], F32, tag="mm", bufs=MM_BUFS, name="ps_v")
            for c in range(NC_DM):
                nc.tensor.matmul(
                    ps_v[:, :],
                    x_sb[("xvT", b)][:, c, ts(tk, 128)],
                    w_sb["wv"][:, c, :],
                    start=(c == 0),
                    stop=(c == NC_DM - 1),
                )
            vs = vspool.tile([128, H, D], BF16, tag="vs")
            nc.vector.tensor_mul(
                vs[:, :, :],
                ps_v[:, :].rearrange("p (h e) -> p h e", h=H),
                rk[:, :].to_broadcast([128, H, D]),
            )
            return {"ke": ke, "vs": vs, "tk": tk}

        def kv_mm(st):
            """Deferred kv accumulation matmuls for one chunk."""
            tk = st["tk"]
            # One PSUM bank holds all 4 head-pair blocks. start_tensor_calc
            # pends-zero the whole 2KB zero region, so only the first matmul
            # starts the group and only the very last stops it.
            for g in range(NPAIR):
                nc.tensor.matmul(
                    kv_ps[:, ds(128 * g, 128)],
                    st["vs"][:, ds(2 * g, 2), :],
                    st["ke"][:, ds(2 * g, 2), :],
                    start=(tk == 0 and g == 0),
                    stop=(tk == NT128 - 1 and g == NPAIR - 1),
                )

        kb_pend = {}  # b -> list of 4 extracted kvblk tiles

        def w2_prep(b):
            """kvblk extraction (DVE) for all 4 head pairs, hoisted to the
            inter's start so the braided W2 matmuls never wait on DVE."""
            kbs = []
            for g in range(NPAIR):
                kb = kbpool.tile([128, 128], BF16, tag="kvblk")
                nc.vector.memset(kb[:, :], 0.0)
                nc.vector.tensor_copy(kb[0:64, 0:64], kv_ps[0:64, ds(128 * g, 64)])
                nc.vector.tensor_copy(
                    kb[64:128, 64:128], kv_ps[64:128, ds(128 * g + 64, 64)]
                )
                kbs.append(kb)
            kb_pend[b] = kbs

        def w2_g(b, g):
            """Merged-Wo matmul for head pair g (kvblk already extracted)."""
            kb = kb_pend[b][g]
            w2_ps = pspool.tile([128, DM], F32, tag="aux", bufs=AUX_BUFS, name="w2_ps")
            nc.tensor.matmul(
                w2_ps[:, :], kb[:, :], w_sb["wo"][:, g, :], start=True, stop=True
            )
            w2 = w2pool.tile([128, DM], BF16, tag="w2")
            nc.scalar.activation(w2[:, :], w2_ps[:, :], COPY)
            w2_sb.setdefault(b, [None] * NPAIR)[g] = w2

        q_state = {}

        def front_q(b, t, c):
            """Q projection matmuls for feature chunk c of token chunk t
            + exp. The pooling matmul is deferred one braid slot (pool_c)
            so the PE never waits on the ACT exp."""
            st = q_state.setdefault((b, t), {})
            ps_q = pspool.tile([128, 512], F32, tag="mm", bufs=MM_BUFS, name="ps_q")
            for k in range(NC_DM):
                nc.tensor.matmul(
                    ps_q[:, :],
                    w_sb["wq"][:, k, ds(128 * c, 128)],
                    x_sb[("xqT", b)][:, k, ds(512 * t, 512)],
                    start=(k == 0),
                    stop=(k == NC_DM - 1),
                )
            qe = qepool.tile([128, 512], BF16, tag="qe")
            nc.scalar.activation(qe[:, :], ps_q[:, :], EXP)
            st.setdefault("qe", []).append(qe)

        def pool_c(b, t, c):
            st = q_state[(b, t)]
            if c == 0:
                st["s_ps"] = pspool.tile([H, 512], F32, tag="sps", bufs=1,
                                         name="s_ps")
            nc.tensor.matmul(
                st["s_ps"][:, :],
                pool8_sb[:, c, :],
                st["qe"][c][:, :],
                start=(c == 0),
                stop=(c == NC_DM - 1),
            )

        def recip(b, t):
            st = q_state[(b, t)]
            rq32 = rqpool.tile([H, 512], F32, tag="rq32")
            nc.vector.reciprocal_approx_fast(rq32[:, :], st["s_ps"][:, :])
            if BC_VIA_DMA:
                # bf16 copy of the reciprocals; broadcast via stride-0 DMA
                rqb = rqpool.tile([H, 512], BF16, tag="rqb")
                nc.vector.tensor_copy(rqb[:, :], rq32[:, :])
                st["rqb"] = rqb
            else:
                rq = rqpool.tile([H, 512], F32R, tag="rq")
                # f32->f32r copy on ACT: keeps the DVE free for the qen muls
                # that gate the output projection
                nc.scalar.activation(rq[:, :], rq32[:, :], COPY)
                st["rq"] = rq

        def bc_c(b, t, c):
            st = q_state[(b, t)]
            if BC_VIA_DMA:
                # replicate rq rows for heads 2c, 2c+1 across their 64
                # partitions with gpsimd software-DGE DMAs (stride-0 source)
                bc = bcpool.tile([128, 512], BF16, tag="bc")
                for i in range(2):
                    src = (
                        st["rqb"][ds(2 * c + i, 1), :]
                        .rearrange("p (x f) -> p x f", x=1)
                        .broadcast_to([1, 64, 512])
                    )
                    nc.gpsimd.dma_start(out=bc[ds(64 * i, 64), :], in_=src)
            else:
                bc = pspool.tile([128, 512], F32, tag="aux", bufs=AUX_BUFS, name="bc")
                nc.tensor.matmul(
                    bc[:, :], exp8_sb[:, ds(128 * c, 128)], st["rq"][:, :],
                    start=True, stop=True,
                )
            qen = qnpool.tile([128, 512], BF16, tag="qen")
            nc.vector.tensor_mul(qen[:, :], st["qe"][c][:, :], bc[:, :])
            st.setdefault("qen", []).append(qen)

        def pso_u(b, t, u):
            st = q_state[(b, t)]
            ps_o = pspool.tile([128, DM], F32, tag="pso", name="ps_o")
            for c in range(NC_DM):
                nc.tensor.matmul(
                    ps_o[:, :],
                    st["qen"][c][:, ds(128 * u, 128)],
                    w2_sb[b][c][:, :],
                    start=(c == 0),
                    stop=(c == NC_DM - 1),
                )
            fin = fpool.tile([128, DM], F32, tag="fin")
            # alternate PSUM evacuation + out DMA between the ACT and SP
            # hwdge queues to split both the copy and the transfer load
            if u % 2 == 0:
                nc.scalar.activation(fin[:, :], ps_o[:, :], COPY)
                nc.scalar.dma_start(
                    out=out_d[b, ds(512 * t + 128 * u, 128), :], in_=fin[:, :]
                )
            else:
                nc.vector.tensor_copy(fin[:, :], ps_o[:, :])
                nc.sync.dma_start(
                    out=out_d[b, ds(512 * t + 128 * u, 128), :], in_=fin[:, :]
                )
            if u == NT512 - 1:
                del q_state[(b, t)]

        def prologue(b):
            # everything pass1(b) needs now, plus the first inter's needs
            dma_x("xkT", b)
            dma_x("xvT", b)
            dma_x("xqT", b)
            dma_x("xkT", 1 - b)
            dma_x("xvT", 1 - b)
            pend = []
            for tk in range(NT128):
                pend.append(chunk_mm(b, tk))
                if len(pend) > 1:
                    kv_mm(pend.pop(0))
            while pend:
                kv_mm(pend.pop(0))

        def inter(bp, bc):
            """pass2 of batch bp braided with pass1 of batch bc.
            chunk slots: t0 fronts carry W2(bp); q_back(t) carries one
            chunk; t1..t3 fronts carry the rest (4 each). kv matmuls
            trail their chunk by two braid slots. x DMAs prefetch for the
            NEXT inter (xq of bc; xk/xv of bp) -- a full inter of lead --
            and are spread across t-groups so out DMAs never queue behind
            more than one 2MB transfer."""
            dma_x("xqT", bc)
            w2_prep(bp)
            pend_kv = []  # chunk states awaiting their kv matmuls

            def do_chunk(i):
                pend_kv.append(chunk_mm(bc, i))

            def flush_kv(depth=KV_DEPTH):
                while len(pend_kv) >= depth:
                    kv_mm(pend_kv.pop(0))

            ci = 0  # next chunk index
            for t in range(NT512):
                for c in range(NC_DM):
                    front_q(bp, t, c)
                    if t == 0:
                        w2_g(bp, c)
                    else:
                        flush_kv()
                        do_chunk(ci)
                        ci += 1
                    # pool matmul lands a braid slot after its qe exp
                    pool_c(bp, t, c)
                recip(bp, t)
                # q_back: the bridging chunk covers the recip->rq chain
                # before the broadcast matmuls need it
                flush_kv()
                do_chunk(ci)
                ci += 1
                bc_c(bp, t, 0)
                bc_c(bp, t, 1)
                bc_c(bp, t, 2)
                bc_c(bp, t, 3)
                pso_u(bp, t, 0)
                flush_kv()
                pso_u(bp, t, 1)
                pso_u(bp, t, 2)
                pso_u(bp, t, 3)
                if t == 0:
                    dma_x("xkT", bp)
                elif t == 1:
                    dma_x("xvT", bp)
            assert ci == NT128
            flush_kv(depth=1)  # drain the tail before the next inter's W2

        prologue(0)
        loop_ctx = (
            tc.For_i(0, loop_n, 1, staggered_reset=True) if loop_n > 1 else None
        )
        if loop_ctx is not None:
            ctx.enter_context(loop_ctx)
        for _rep in range(reps):
            inter(0, 1)
            inter(1, 0)
    nc.compile()
    return nc


def make_const_inputs():
    pool8 = np.zeros((128, NC_DM, H), np.float32)
    for p in range(128):
        for c in range(NC_DM):
            pool8[p, c, 2 * c + p // 64] = 1.0
    exp8 = np.zeros((H, DM), np.float32)
    for c in range(NC_DM):
        for j in range(128):
            exp8[2 * c + j // 64, 128 * c + j] = 1.0
    return pool8.astype(NPBF16), exp8


def make_in_maps(xq, xk, xv, Wq, Wk, Wv, Wo):
    pool8, exp8 = make_const_inputs()
    scale = np.float32(D**-0.5)
    consts = {
        "wq": np.asarray(Wq, np.float32).astype(NPBF16),
        "wk": np.asarray(Wk, np.float32).astype(NPBF16),
        "wv": np.asarray(Wv, np.float32).astype(NPBF16),
        "wo": (np.asarray(Wo, np.float32) * scale).astype(NPBF16),
        "pool8": pool8,
        "exp8": exp8,
    }

    def prep(x, sl):
        xt = np.asarray(x[sl], np.float32).transpose(0, 2, 1)
        return np.ascontiguousarray(xt).astype(NPBF16)

    in_maps = []
    for core in range(NCORES):
        sl = slice(MB * core, MB * (core + 1))
        m = dict(consts)
        m["xqT"] = prep(xq, sl)
        m["xkT"] = prep(xk, sl)
        m["xvT"] = prep(xv, sl)
        in_maps.append(m)
    return in_maps


_NC = None


def kernel(xq, xk, xv, Wq, Wk, Wv, Wo, bo):
    global _NC
    if _NC is None:
        _NC = build_program()
    in_maps = make_in_maps(xq, xk, xv, Wq, Wk, Wv, Wo)
    res = run_bass_kernel_spmd(_NC, in_maps, core_ids=list(range(NCORES)))
    out = np.concatenate([res.results[i]["out"] for i in range(NCORES)], axis=0)
    out += np.asarray(bo, np.float32)[None, None, :]
    return out
